# revision 49
# baseline (speedup 1.0000x reference)
"""RNNT joint log_softmax kernel for Trainium2 (Bass/Tile), 8-core SPMD.

out[b,t,u,v] = log_softmax(f[b,t,v] + g[b,u,v], axis=v)

Sharding: 8 shards over (b, t-half): core i handles b=i//2, t in [128*(i%2), ...).
lse trick: exp(f+g) = exp(f)*exp(g), so lse[t,u] = ln(exp(f) @ exp(g)^T).

Main loop (partitions = t, loop over u): moving tile B = [F rows | the group's
4 g-rows], constant stationary W_j = diag + g-select row, so ONE matmul pair
per u computes F[t,v] + g_u[v] for 124 t-rows in PSUM. The epilogue applies
-lse[:,u] as a per-partition scalar while downcasting PSUM->fp16; DVE
(tensor_scalar_sub) takes even u's, ACT (activation bias) odd u's — separate
psum/stage tiles per engine so the tile-granular dependency tracker never
serializes them. GPSIMD cannot touch PSUM; Pool only does SBUF-side work.
Inputs land as f16 (host-converted) and output is written fp16 (rel err
~6e-3 << 2e-2 gate), halving HBM traffic; host upcasts. All ACT functions
(Exp/Ln/Identity) are steered into the one table set containing them so a
single LoadActFuncSet runs off the critical path. Engine APs must start at
partition 0/32/64/96, so the 4 g-rows sit at 96:100 (written by DMA, which
has no such limit) and leftover t-rows 124..127 go through a one-hot
broadcast path (lse added via a K=1 ones-row matmul), interleaved mid-loop.
"""

import numpy as np

B, T, U, V = 4, 256, 128, 1024
TSH = 128          # t-shard per core
NCORES = 8
TMAIN = 124        # t-rows handled by the B-tile trick per matmul
NG = 4             # g-rows resident in B / u's per group
NGRP = U // NG     # 32 groups
NB = 4             # B-tile ring

_nc_cache = {}


def _build(tag="main"):
    if tag in _nc_cache:
        return _nc_cache[tag]
    from contextlib import ExitStack

    import concourse.bacc as bacc
    import concourse.tile as tile
    from concourse import mybir

    f32 = mybir.dt.float32
    f16 = mybir.dt.float16
    AF = mybir.ActivationFunctionType

    # Steer Exp/Ln/Identity/Copy into the single table set holding them all
    # ("natural_log_exp_and_others") so only one LoadActFuncSet is emitted.
    # Set indices/IDs are unchanged and the chosen set genuinely contains
    # these functions, so the emitted BIR stays valid for walrus.
    _orig_tables = bacc.get_activation_tables

    def _steered_tables(arch):
        out = {}
        for name, funcs in _orig_tables(arch).items():
            funcs = set(funcs)
            if name != "natural_log_exp_and_others":
                funcs.discard(AF.Exp)
                funcs.discard(AF.Ln)
                funcs.discard(AF.Identity)
                funcs.discard(AF.Copy)
            out[name] = funcs
        return out

    nc = bacc.Bacc("TRN2", debug=False, num_devices=NCORES)
    f_d = nc.dram_tensor("f_sh", [TSH, V], f16, kind="ExternalInput").ap()
    g_d = nc.dram_tensor("g_sh", [U, V], f16, kind="ExternalInput").ap()
    eye_d = nc.dram_tensor("eye16", [128, 128], f16, kind="ExternalInput").ap()
    w8_d = nc.dram_tensor("w8", [128, NG * 128], f16, kind="ExternalInput").ap()
    ones_d = nc.dram_tensor("ones_row", [1, V], f16, kind="ExternalInput").ap()
    out_d = nc.dram_tensor("out_sh", [TSH, U, V], f16, kind="ExternalOutput").ap()

    with tile.TileContext(nc) as tc, ExitStack() as ctx:
        const_pool = ctx.enter_context(tc.tile_pool(name="const", bufs=1))

        F16 = const_pool.tile([128, V], f16)
        G16 = const_pool.tile([128, V], f16)
        eye16 = const_pool.tile([128, 128], f16)
        W8 = const_pool.tile([128, NG * 128], f16)
        scr = const_pool.tile([128, 128], f16)
        ones1 = const_pool.tile([1, V], f16)
        nc.vector.memset(scr[:], 0.0)

        # B ring layout (engine APs must start at partition 0/32/64/96):
        # rows 0:96 = F[0:96], rows 96:100 = the group's 4 g-rows,
        # rows 100:128 = F[96:124]. Seeds/tails go via DMA (no partition
        # limits there); B0's seeds ride the Pool SWDGE queue to keep the
        # SP HWDGE queue short early on.
        Bt = [const_pool.tile([128, V], f16, name=f"Bt{q}") for q in range(NB)]
        nc.sync.dma_start(eye16[:], eye_d[:])
        nc.sync.dma_start(F16[:], f_d[:])
        nc.sync.dma_start(G16[:], g_d[:])
        nc.sync.dma_start(W8[:], w8_d[:])
        nc.sync.dma_start(ones1[:], ones_d[:])
        nc.gpsimd.dma_start(Bt[0][96:100, :], g_d[0:NG, :])
        nc.gpsimd.dma_start(Bt[0][100:128, :], f_d[96:TMAIN, :])
        for q in range(NB):
            nc.vector.tensor_copy(Bt[q][0:96, :], F16[0:96, :])

        # --- preamble: lse (t-orientation) via exp-transpose-matmul ---
        EfT = const_pool.tile([128, V], f16)   # col block c: [v-chunk, t]
        EgT = const_pool.tile([128, V], f16)
        neg_lse16 = const_pool.tile([128, 128], f16)    # [t, u] (leftovers)
        with tc.tile_pool(name="psum_pre", bufs=2, space="PSUM") as pre_psum, \
             tc.tile_pool(name="psum_s", bufs=1, space="PSUM") as s_pool:
            # PE p-state warmup while input DMAs land (results unused);
            # scr needs no DMA, so PE ramps from t~0 into the real transposes
            warm = pre_psum.tile([128, 128], f16, tag="warm")
            for _ in range(16):
                nc.tensor.transpose(warm[:], scr[:], scr[:])
            for src, dst in ((F16, EfT), (G16, EgT)):
                tp = pre_psum.tile([128, V], f16, tag="tp")
                for c in range(8):
                    nc.tensor.transpose(
                        tp[:, 128 * c:128 * (c + 1)],
                        src[:, 128 * c:128 * (c + 1)], eye16[:])
                nc.scalar.activation(dst[:], tp[:], AF.Exp)
            lse_t32 = const_pool.tile([128, 128], f32)
            neg_lse32 = const_pool.tile([128, 128], f32)
            s_ps = s_pool.tile([128, 128], f32, tag="s")
            for c in range(8):
                sl = slice(128 * c, 128 * (c + 1))
                nc.tensor.matmul(s_ps[:], EfT[:, sl], EgT[:, sl],
                                 start=(c == 0), stop=(c == 7))
            # u-slice the Ln/negate so the first groups unblock earliest
            nc.scalar.activation(lse_t32[:, 0:32], s_ps[:, 0:32], AF.Ln)
            nc.gpsimd.tensor_scalar_mul(
                neg_lse32[:, 0:32], lse_t32[:, 0:32], -1.0)
            nc.scalar.activation(lse_t32[:, 32:], s_ps[:, 32:], AF.Ln)
            nc.gpsimd.tensor_scalar_mul(
                neg_lse32[:, 32:], lse_t32[:, 32:], -1.0)
            nc.gpsimd.tensor_scalar_mul(neg_lse16[:], lse_t32[:], -1.0)
        # remaining B seeds once Pool's queue is clear
        for q in range(1, NB):
            nc.gpsimd.dma_start(Bt[q][96:100, :], g_d[NG * q:NG * (q + 1), :])
            nc.gpsimd.dma_start(Bt[q][100:128, :], f_d[96:TMAIN, :])

        # leftover rows' -lse flattened into partition 0 (DMA may cross
        # partitions freely) for the K=1 ones-row matmul stationary
        L0T = const_pool.tile([1, (TSH - TMAIN) * 128], f16)
        nc.sync.dma_start(L0T[0:1, :], neg_lse16[TMAIN:TSH, :])

        # --- main loop: NGRP groups x NG u's; leftover t-rows interleaved ---
        out_pool = ctx.enter_context(tc.tile_pool(name="out", bufs=6))
        lo_pool = ctx.enter_context(tc.tile_pool(name="lo", bufs=2))
        nlo = TSH - TMAIN
        lo_every = NGRP // nlo
        with tc.tile_pool(name="psum_b", bufs=4, space="PSUM") as psum_b:
            for m in range(NGRP):
                Bb = Bt[m % NB]
                # separate stage tiles per engine: no shared writer tiles,
                # so no cross-engine serialization through the tracker
                stageD = out_pool.tile([TMAIN, 2, V], f16, tag="sD")
                stageA = out_pool.tile([TMAIN, 2, V], f16, tag="sA")
                for j in range(NG):
                    u = NG * m + j
                    Wj = W8[:, 128 * j:128 * j + TMAIN]
                    pb = psum_b.tile([128, V], f32, tag="pb")
                    for c2 in range(2):
                        bsl = slice(512 * c2, 512 * (c2 + 1))
                        nc.tensor.matmul(
                            pb[:TMAIN, bsl], Wj, Bb[:, bsl],
                            start=True, stop=True)
                    # epilogue: PSUM->f16 downcast with -lse[:,u] applied as
                    # a per-partition scalar; DVE takes even u's, ACT odd
                    if j % 2 == 0:
                        nc.vector.tensor_scalar_sub(
                            stageD[:, j // 2, :], pb[:TMAIN, :],
                            lse_t32[:TMAIN, u:u + 1])
                    else:
                        nc.scalar.activation(
                            stageA[:, j // 2, :], pb[:TMAIN, :],
                            AF.Identity, bias=neg_lse32[:TMAIN, u:u + 1])
                # refresh this B tile's g-rows for group m+NB
                if m + NB < NGRP:
                    nc.sync.dma_start(
                        Bt[m % NB][96:100, :],
                        G16[NG * (m + NB):NG * (m + NB + 1), :],
                    )
                u0 = NG * m
                nc.sync.dma_start(
                    out_d[0:TMAIN, u0:u0 + NG:2, :], stageD[:])
                nc.sync.dma_start(
                    out_d[0:TMAIN, u0 + 1:u0 + NG:2, :], stageA[:])

                # one leftover t-row (one-hot f_t broadcast, u-partitions;
                # lse added via a K=1 ones-row matmul), interleaved
                if m % lo_every == lo_every // 2 - 1:
                    t = TMAIN + m // lo_every
                    lse_row = L0T[0:1, 128 * (t - TMAIN):128 * (t - TMAIN + 1)]
                    pb2 = psum_b.tile([128, V], f32, tag="pb")
                    onehot = eye16[:, t:t + 1].broadcast_to([128, 128])
                    for c2 in range(2):
                        sl = slice(512 * c2, 512 * (c2 + 1))
                        nc.tensor.matmul(pb2[:, sl], onehot, F16[:, sl],
                                         start=True, stop=False)
                        nc.tensor.matmul(pb2[:, sl], eye16[:], G16[:, sl],
                                         start=False, stop=False)
                        nc.tensor.matmul(pb2[:, sl], lse_row,
                                         ones1[:, sl], start=False, stop=True)
                    stage2 = lo_pool.tile([128, V], f16)
                    nc.vector.tensor_copy(stage2[:, 0:512], pb2[:, 0:512])
                    nc.scalar.activation(stage2[:, 512:], pb2[:, 512:],
                                         AF.Copy)
                    nc.sync.dma_start(out_d[t, :, :], stage2[:])

    bacc.get_activation_tables = _steered_tables
    try:
        nc.compile()
    finally:
        bacc.get_activation_tables = _orig_tables
    _nc_cache[tag] = nc
    return nc


def _consts():
    eye16 = np.eye(128, dtype=np.float16)
    # B-row map: rows 0:96 = F[0:96], rows 96:100 = g-rows,
    # rows 100:128 = F[96:124]
    w8 = np.zeros((128, NG * 128), dtype=np.float16)
    for j in range(NG):
        blk = w8[:, 128 * j:128 * (j + 1)]
        for k in range(96):
            blk[k, k] = 1.0
        for k in range(100, 128):
            blk[k, k - 4] = 1.0
        for t in range(TMAIN):
            blk[96 + j, t] = 1.0
    return eye16, w8


def _in_maps(f, g):
    eye16, w8 = _consts()
    ones_row = np.ones((1, V), dtype=np.float16)
    f16 = f.astype(np.float16)
    g16 = g.astype(np.float16)
    maps = []
    for i in range(NCORES):
        b, h = divmod(i, 2)
        maps.append({
            "f_sh": np.ascontiguousarray(f16[b, h * TSH:(h + 1) * TSH]),
            "g_sh": np.ascontiguousarray(g16[b]),
            "eye16": eye16,
            "w8": w8,
            "ones_row": ones_row,
        })
    return maps


def _gather(results):
    out = np.empty((B, T, U, V), np.float32)
    for i in range(NCORES):
        b, h = divmod(i, 2)
        out[b, h * TSH:(h + 1) * TSH] = np.asarray(
            results[i]["out_sh"], dtype=np.float32)
    return out


def kernel(**inputs):
    from concourse.bass_utils import run_bass_kernel_spmd

    f = np.asarray(inputs["f"], np.float32)
    g = np.asarray(inputs["g"], np.float32)
    nc = _build()
    res = run_bass_kernel_spmd(nc, _in_maps(f, g), core_ids=list(range(NCORES)))
    return _gather(res.results)


# revision 52
# speedup vs baseline: 1.0041x; 1.0041x over previous
"""RNNT joint log_softmax kernel for Trainium2 (Bass/Tile), 8-core SPMD.

out[b,t,u,v] = log_softmax(f[b,t,v] + g[b,u,v], axis=v)

Sharding: 8 shards over (b, t-half): core i handles b=i//2, t in [128*(i%2), ...).
lse trick: exp(f+g) = exp(f)*exp(g), so lse[t,u] = ln(exp(f) @ exp(g)^T).

Main loop (partitions = t, loop over u): moving tile B = [F rows | the group's
4 g-rows], constant stationary W_j = diag + g-select row, so ONE matmul pair
per u computes F[t,v] + g_u[v] for 124 t-rows in PSUM. The epilogue applies
-lse[:,u] as a per-partition scalar while downcasting PSUM->fp16; DVE
(tensor_scalar_sub) takes even u's, ACT (activation bias) odd u's — separate
psum/stage tiles per engine so the tile-granular dependency tracker never
serializes them. GPSIMD cannot touch PSUM; Pool only does SBUF-side work.
Inputs land as f16 (host-converted) and output is written fp16 (rel err
~6e-3 << 2e-2 gate), halving HBM traffic; host upcasts. All ACT functions
(Exp/Ln/Identity) are steered into the one table set containing them so a
single LoadActFuncSet runs off the critical path. Engine APs must start at
partition 0/32/64/96, so the 4 g-rows sit at 96:100 (written by DMA, which
has no such limit) and leftover t-rows 124..127 go through a one-hot
broadcast path (lse added via a K=1 ones-row matmul), interleaved mid-loop.
"""

import numpy as np

B, T, U, V = 4, 256, 128, 1024
TSH = 128          # t-shard per core
NCORES = 8
TMAIN = 124        # t-rows handled by the B-tile trick per matmul
NG = 4             # g-rows resident in B / u's per group
NGRP = U // NG     # 32 groups
NB = 4             # B-tile ring

_nc_cache = {}


def _build(tag="main"):
    if tag in _nc_cache:
        return _nc_cache[tag]
    from contextlib import ExitStack

    import concourse.bacc as bacc
    import concourse.tile as tile
    from concourse import mybir

    f32 = mybir.dt.float32
    f16 = mybir.dt.float16
    AF = mybir.ActivationFunctionType

    # Steer Exp/Ln/Identity/Copy into the single table set holding them all
    # ("natural_log_exp_and_others") so only one LoadActFuncSet is emitted.
    # Set indices/IDs are unchanged and the chosen set genuinely contains
    # these functions, so the emitted BIR stays valid for walrus.
    _orig_tables = bacc.get_activation_tables

    def _steered_tables(arch):
        out = {}
        for name, funcs in _orig_tables(arch).items():
            funcs = set(funcs)
            if name != "natural_log_exp_and_others":
                funcs.discard(AF.Exp)
                funcs.discard(AF.Ln)
                funcs.discard(AF.Identity)
                funcs.discard(AF.Copy)
            out[name] = funcs
        return out

    nc = bacc.Bacc("TRN2", debug=False, num_devices=NCORES)
    f_d = nc.dram_tensor("f_sh", [TSH, V], f16, kind="ExternalInput").ap()
    g_d = nc.dram_tensor("g_sh", [U, V], f16, kind="ExternalInput").ap()
    eye_d = nc.dram_tensor("eye16", [128, 128], f16, kind="ExternalInput").ap()
    w8_d = nc.dram_tensor("w8", [128, NG * 128], f16, kind="ExternalInput").ap()
    ones_d = nc.dram_tensor("ones_row", [1, V], f16, kind="ExternalInput").ap()
    out_d = nc.dram_tensor("out_sh", [TSH, U, V], f16, kind="ExternalOutput").ap()

    with tile.TileContext(nc) as tc, ExitStack() as ctx:
        const_pool = ctx.enter_context(tc.tile_pool(name="const", bufs=1))

        F16 = const_pool.tile([128, V], f16)
        G16 = const_pool.tile([128, V], f16)
        eye16 = const_pool.tile([128, 128], f16)
        W8 = const_pool.tile([128, NG * 128], f16)
        scr = const_pool.tile([128, 128], f16)
        ones1 = const_pool.tile([1, V], f16)
        nc.vector.memset(scr[:], 0.0)

        # B ring layout (engine APs must start at partition 0/32/64/96):
        # rows 0:96 = F[0:96], rows 96:100 = the group's 4 g-rows,
        # rows 100:128 = F[96:124]. Seeds/tails go via DMA (no partition
        # limits there); B0's seeds ride the Pool SWDGE queue to keep the
        # SP HWDGE queue short early on.
        Bt = [const_pool.tile([128, V], f16, name=f"Bt{q}") for q in range(NB)]
        nc.sync.dma_start(F16[:], f_d[:])
        nc.sync.dma_start(eye16[:], eye_d[:])
        nc.sync.dma_start(G16[:], g_d[:])
        nc.sync.dma_start(W8[:], w8_d[:])
        nc.sync.dma_start(ones1[:], ones_d[:])
        nc.gpsimd.dma_start(Bt[0][96:100, :], g_d[0:NG, :])
        nc.gpsimd.dma_start(Bt[0][100:128, :], f_d[96:TMAIN, :])
        for q in range(NB):
            nc.vector.tensor_copy(Bt[q][0:96, :], F16[0:96, :])

        # --- preamble: lse (t-orientation) via exp-transpose-matmul ---
        EfT = const_pool.tile([128, V], f16)   # col block c: [v-chunk, t]
        EgT = const_pool.tile([128, V], f16)
        neg_lse16 = const_pool.tile([128, 128], f16)    # [t, u] (leftovers)
        with tc.tile_pool(name="psum_pre", bufs=2, space="PSUM") as pre_psum, \
             tc.tile_pool(name="psum_s", bufs=1, space="PSUM") as s_pool:
            # PE p-state warmup while input DMAs land (results unused);
            # scr needs no DMA, so PE ramps from t~0 into the real transposes
            warm = pre_psum.tile([128, 128], f16, tag="warm")
            for _ in range(16):
                nc.tensor.transpose(warm[:], scr[:], scr[:])
            for src, dst in ((F16, EfT), (G16, EgT)):
                tp = pre_psum.tile([128, V], f16, tag="tp")
                for c in range(8):
                    nc.tensor.transpose(
                        tp[:, 128 * c:128 * (c + 1)],
                        src[:, 128 * c:128 * (c + 1)], eye16[:])
                nc.scalar.activation(dst[:], tp[:], AF.Exp)
            lse_t32 = const_pool.tile([128, 128], f32)
            neg_lse32 = const_pool.tile([128, 128], f32)
            s_ps = s_pool.tile([128, 128], f32, tag="s")
            for c in range(8):
                sl = slice(128 * c, 128 * (c + 1))
                nc.tensor.matmul(s_ps[:], EfT[:, sl], EgT[:, sl],
                                 start=(c == 0), stop=(c == 7))
            # u-slice the Ln/negate so the first groups unblock earliest
            nc.scalar.activation(lse_t32[:, 0:32], s_ps[:, 0:32], AF.Ln)
            nc.gpsimd.tensor_scalar_mul(
                neg_lse32[:, 0:32], lse_t32[:, 0:32], -1.0)
            nc.scalar.activation(lse_t32[:, 32:], s_ps[:, 32:], AF.Ln)
            nc.gpsimd.tensor_scalar_mul(
                neg_lse32[:, 32:], lse_t32[:, 32:], -1.0)
            nc.gpsimd.tensor_scalar_mul(neg_lse16[:], lse_t32[:], -1.0)
        # remaining B seeds once Pool's queue is clear
        for q in range(1, NB):
            nc.gpsimd.dma_start(Bt[q][96:100, :], g_d[NG * q:NG * (q + 1), :])
            nc.gpsimd.dma_start(Bt[q][100:128, :], f_d[96:TMAIN, :])

        # leftover rows' -lse flattened into partition 0 (DMA may cross
        # partitions freely) for the K=1 ones-row matmul stationary
        L0T = const_pool.tile([1, (TSH - TMAIN) * 128], f16)
        nc.sync.dma_start(L0T[0:1, :], neg_lse16[TMAIN:TSH, :])

        # --- main loop: NGRP groups x NG u's; leftover t-rows interleaved ---
        out_pool = ctx.enter_context(tc.tile_pool(name="out", bufs=6))
        lo_pool = ctx.enter_context(tc.tile_pool(name="lo", bufs=2))
        nlo = TSH - TMAIN
        lo_every = NGRP // nlo
        with tc.tile_pool(name="psum_b", bufs=4, space="PSUM") as psum_b:
            for m in range(NGRP):
                Bb = Bt[m % NB]
                # separate stage tiles per engine: no shared writer tiles,
                # so no cross-engine serialization through the tracker
                stageD = out_pool.tile([TMAIN, 2, V], f16, tag="sD")
                stageA = out_pool.tile([TMAIN, 2, V], f16, tag="sA")
                for j in range(NG):
                    u = NG * m + j
                    Wj = W8[:, 128 * j:128 * j + TMAIN]
                    pb = psum_b.tile([128, V], f32, tag="pb")
                    for c2 in range(2):
                        bsl = slice(512 * c2, 512 * (c2 + 1))
                        nc.tensor.matmul(
                            pb[:TMAIN, bsl], Wj, Bb[:, bsl],
                            start=True, stop=True)
                    # epilogue: PSUM->f16 downcast with -lse[:,u] applied as
                    # a per-partition scalar; DVE takes even u's, ACT odd
                    if j % 2 == 0:
                        nc.vector.tensor_scalar_sub(
                            stageD[:, j // 2, :], pb[:TMAIN, :],
                            lse_t32[:TMAIN, u:u + 1])
                    else:
                        nc.scalar.activation(
                            stageA[:, j // 2, :], pb[:TMAIN, :],
                            AF.Identity, bias=neg_lse32[:TMAIN, u:u + 1])
                # refresh this B tile's g-rows for group m+NB
                if m + NB < NGRP:
                    nc.sync.dma_start(
                        Bt[m % NB][96:100, :],
                        G16[NG * (m + NB):NG * (m + NB + 1), :],
                    )
                u0 = NG * m
                nc.sync.dma_start(
                    out_d[0:TMAIN, u0:u0 + NG:2, :], stageD[:])
                nc.sync.dma_start(
                    out_d[0:TMAIN, u0 + 1:u0 + NG:2, :], stageA[:])

                # one leftover t-row (one-hot f_t broadcast, u-partitions;
                # lse added via a K=1 ones-row matmul), interleaved
                if m % lo_every == lo_every // 2 - 1:
                    t = TMAIN + m // lo_every
                    lse_row = L0T[0:1, 128 * (t - TMAIN):128 * (t - TMAIN + 1)]
                    pb2 = psum_b.tile([128, V], f32, tag="pb")
                    onehot = eye16[:, t:t + 1].broadcast_to([128, 128])
                    for c2 in range(2):
                        sl = slice(512 * c2, 512 * (c2 + 1))
                        nc.tensor.matmul(pb2[:, sl], onehot, F16[:, sl],
                                         start=True, stop=False)
                        nc.tensor.matmul(pb2[:, sl], eye16[:], G16[:, sl],
                                         start=False, stop=False)
                        nc.tensor.matmul(pb2[:, sl], lse_row,
                                         ones1[:, sl], start=False, stop=True)
                    stage2 = lo_pool.tile([128, V], f16)
                    nc.vector.tensor_copy(stage2[:, 0:512], pb2[:, 0:512])
                    nc.scalar.activation(stage2[:, 512:], pb2[:, 512:],
                                         AF.Copy)
                    nc.sync.dma_start(out_d[t, :, :], stage2[:])

    bacc.get_activation_tables = _steered_tables
    try:
        nc.compile()
    finally:
        bacc.get_activation_tables = _orig_tables
    _nc_cache[tag] = nc
    return nc


def _consts():
    eye16 = np.eye(128, dtype=np.float16)
    # B-row map: rows 0:96 = F[0:96], rows 96:100 = g-rows,
    # rows 100:128 = F[96:124]
    w8 = np.zeros((128, NG * 128), dtype=np.float16)
    for j in range(NG):
        blk = w8[:, 128 * j:128 * (j + 1)]
        for k in range(96):
            blk[k, k] = 1.0
        for k in range(100, 128):
            blk[k, k - 4] = 1.0
        for t in range(TMAIN):
            blk[96 + j, t] = 1.0
    return eye16, w8


def _in_maps(f, g):
    eye16, w8 = _consts()
    ones_row = np.ones((1, V), dtype=np.float16)
    f16 = f.astype(np.float16)
    g16 = g.astype(np.float16)
    maps = []
    for i in range(NCORES):
        b, h = divmod(i, 2)
        maps.append({
            "f_sh": np.ascontiguousarray(f16[b, h * TSH:(h + 1) * TSH]),
            "g_sh": np.ascontiguousarray(g16[b]),
            "eye16": eye16,
            "w8": w8,
            "ones_row": ones_row,
        })
    return maps


def _gather(results):
    out = np.empty((B, T, U, V), np.float32)
    for i in range(NCORES):
        b, h = divmod(i, 2)
        out[b, h * TSH:(h + 1) * TSH] = np.asarray(
            results[i]["out_sh"], dtype=np.float32)
    return out


def kernel(**inputs):
    from concourse.bass_utils import run_bass_kernel_spmd

    f = np.asarray(inputs["f"], np.float32)
    g = np.asarray(inputs["g"], np.float32)
    nc = _build()
    res = run_bass_kernel_spmd(nc, _in_maps(f, g), core_ids=list(range(NCORES)))
    return _gather(res.results)


# revision 54
# speedup vs baseline: 1.0080x; 1.0039x over previous
"""RNNT joint log_softmax kernel for Trainium2 (Bass/Tile), 8-core SPMD.

out[b,t,u,v] = log_softmax(f[b,t,v] + g[b,u,v], axis=v)

Sharding: 8 shards over (b, t-half): core i handles b=i//2, t in [128*(i%2), ...).
lse trick: exp(f+g) = exp(f)*exp(g), so lse[t,u] = ln(exp(f) @ exp(g)^T).

Main loop (partitions = t, loop over u): moving tile B = [F rows | the group's
4 g-rows], constant stationary W_j = diag + g-select row, so ONE matmul pair
per u computes F[t,v] + g_u[v] for 124 t-rows in PSUM. The epilogue applies
-lse[:,u] as a per-partition scalar while downcasting PSUM->fp16; DVE
(tensor_scalar_sub) takes even u's, ACT (activation bias) odd u's — separate
psum/stage tiles per engine so the tile-granular dependency tracker never
serializes them. GPSIMD cannot touch PSUM; Pool only does SBUF-side work.
Inputs land as f16 (host-converted) and output is written fp16 (rel err
~6e-3 << 2e-2 gate), halving HBM traffic; host upcasts. All ACT functions
(Exp/Ln/Identity) are steered into the one table set containing them so a
single LoadActFuncSet runs off the critical path. Engine APs must start at
partition 0/32/64/96, so the 4 g-rows sit at 96:100 (written by DMA, which
has no such limit) and leftover t-rows 124..127 go through a one-hot
broadcast path (lse added via a K=1 ones-row matmul), interleaved mid-loop.
"""

import numpy as np

B, T, U, V = 4, 256, 128, 1024
TSH = 128          # t-shard per core
NCORES = 8
TMAIN = 124        # t-rows handled by the B-tile trick per matmul
NG = 4             # g-rows resident in B / u's per group
NGRP = U // NG     # 32 groups
NB = 4             # B-tile ring

_nc_cache = {}


def _build(tag="main"):
    if tag in _nc_cache:
        return _nc_cache[tag]
    from contextlib import ExitStack

    import concourse.bacc as bacc
    import concourse.tile as tile
    from concourse import mybir

    f32 = mybir.dt.float32
    f16 = mybir.dt.float16
    AF = mybir.ActivationFunctionType

    # Steer Exp/Ln/Identity/Copy into the single table set holding them all
    # ("natural_log_exp_and_others") so only one LoadActFuncSet is emitted.
    # Set indices/IDs are unchanged and the chosen set genuinely contains
    # these functions, so the emitted BIR stays valid for walrus.
    _orig_tables = bacc.get_activation_tables

    def _steered_tables(arch):
        out = {}
        for name, funcs in _orig_tables(arch).items():
            funcs = set(funcs)
            if name != "natural_log_exp_and_others":
                funcs.discard(AF.Exp)
                funcs.discard(AF.Ln)
                funcs.discard(AF.Identity)
                funcs.discard(AF.Copy)
            out[name] = funcs
        return out

    nc = bacc.Bacc("TRN2", debug=False, num_devices=NCORES)
    f_d = nc.dram_tensor("f_sh", [TSH, V], f16, kind="ExternalInput").ap()
    g_d = nc.dram_tensor("g_sh", [U, V], f16, kind="ExternalInput").ap()
    eye_d = nc.dram_tensor("eye16", [128, 128], f16, kind="ExternalInput").ap()
    w8_d = nc.dram_tensor("w8", [128, NG * 128], f16, kind="ExternalInput").ap()
    ones_d = nc.dram_tensor("ones_row", [1, V], f16, kind="ExternalInput").ap()
    out_d = nc.dram_tensor("out_sh", [TSH, U, V], f16, kind="ExternalOutput").ap()

    with tile.TileContext(nc) as tc, ExitStack() as ctx:
        const_pool = ctx.enter_context(tc.tile_pool(name="const", bufs=1))

        F16 = const_pool.tile([128, V], f16)
        G16 = const_pool.tile([128, V], f16)
        eye16 = const_pool.tile([128, 128], f16)
        W8 = const_pool.tile([128, NG * 128], f16)
        scr = const_pool.tile([128, 128], f16)
        ones1 = const_pool.tile([1, V], f16)
        nc.vector.memset(scr[:], 0.0)

        # B ring layout (engine APs must start at partition 0/32/64/96):
        # rows 0:96 = F[0:96], rows 96:100 = the group's 4 g-rows,
        # rows 100:128 = F[96:124]. Seeds/tails go via DMA (no partition
        # limits there); B0's seeds ride the Pool SWDGE queue to keep the
        # SP HWDGE queue short early on.
        Bt = [const_pool.tile([128, V], f16, name=f"Bt{q}") for q in range(NB)]
        nc.sync.dma_start(F16[:], f_d[:])
        nc.sync.dma_start(eye16[:], eye_d[:])
        nc.sync.dma_start(G16[:], g_d[:])
        nc.sync.dma_start(W8[:], w8_d[:])
        nc.sync.dma_start(ones1[:], ones_d[:])
        nc.gpsimd.dma_start(Bt[0][96:100, :], g_d[0:NG, :])
        nc.gpsimd.dma_start(Bt[0][100:128, :], f_d[96:TMAIN, :])
        for q in range(NB):
            nc.vector.tensor_copy(Bt[q][0:96, :], F16[0:96, :])

        # --- preamble: lse (t-orientation) via exp-transpose-matmul ---
        EfT = const_pool.tile([128, V], f16)   # col block c: [v-chunk, t]
        EgT = const_pool.tile([128, V], f16)
        neg_lse16 = const_pool.tile([128, 128], f16)    # [t, u] (leftovers)
        with tc.tile_pool(name="psum_pre", bufs=2, space="PSUM") as pre_psum, \
             tc.tile_pool(name="psum_s", bufs=1, space="PSUM") as s_pool:
            # PE p-state warmup while input DMAs land (results unused);
            # scr needs no DMA, so PE ramps from t~0 into the real transposes
            warm = pre_psum.tile([128, 128], f16, tag="warm")
            for _ in range(24):
                nc.tensor.transpose(warm[:], scr[:], scr[:])
            for src, dst in ((F16, EfT), (G16, EgT)):
                tp = pre_psum.tile([128, V], f16, tag="tp")
                for c in range(8):
                    nc.tensor.transpose(
                        tp[:, 128 * c:128 * (c + 1)],
                        src[:, 128 * c:128 * (c + 1)], eye16[:])
                nc.scalar.activation(dst[:], tp[:], AF.Exp)
            lse_t32 = const_pool.tile([128, 128], f32)
            neg_lse32 = const_pool.tile([128, 128], f32)
            s_a = s_pool.tile([128, 32], f32, tag="sa")
            s_b = s_pool.tile([128, 96], f32, tag="sb")
            for c in range(8):
                sl = slice(128 * c, 128 * (c + 1))
                nc.tensor.matmul(s_a[:], EfT[:, sl], EgT[:, sl][:, 0:32],
                                 start=(c == 0), stop=(c == 7))
            nc.scalar.activation(lse_t32[:, 0:32], s_a[:], AF.Ln)
            nc.gpsimd.tensor_scalar_mul(
                neg_lse32[:, 0:32], lse_t32[:, 0:32], -1.0)
            for c in range(8):
                sl = slice(128 * c, 128 * (c + 1))
                nc.tensor.matmul(s_b[:], EfT[:, sl], EgT[:, sl][:, 32:128],
                                 start=(c == 0), stop=(c == 7))
            nc.scalar.activation(lse_t32[:, 32:], s_b[:], AF.Ln)
            nc.gpsimd.tensor_scalar_mul(
                neg_lse32[:, 32:], lse_t32[:, 32:], -1.0)
            nc.gpsimd.tensor_scalar_mul(neg_lse16[:], lse_t32[:], -1.0)
        # remaining B seeds once Pool's queue is clear
        for q in range(1, NB):
            nc.gpsimd.dma_start(Bt[q][96:100, :], g_d[NG * q:NG * (q + 1), :])
            nc.gpsimd.dma_start(Bt[q][100:128, :], f_d[96:TMAIN, :])

        # leftover rows' -lse flattened into partition 0 (DMA may cross
        # partitions freely) for the K=1 ones-row matmul stationary
        L0T = const_pool.tile([1, (TSH - TMAIN) * 128], f16)
        nc.sync.dma_start(L0T[0:1, :], neg_lse16[TMAIN:TSH, :])

        # --- main loop: NGRP groups x NG u's; leftover t-rows interleaved ---
        out_pool = ctx.enter_context(tc.tile_pool(name="out", bufs=6))
        lo_pool = ctx.enter_context(tc.tile_pool(name="lo", bufs=2))
        nlo = TSH - TMAIN
        lo_every = NGRP // nlo
        with tc.tile_pool(name="psum_b", bufs=4, space="PSUM") as psum_b:
            for m in range(NGRP):
                Bb = Bt[m % NB]
                # separate stage tiles per engine: no shared writer tiles,
                # so no cross-engine serialization through the tracker
                stageD = out_pool.tile([TMAIN, 2, V], f16, tag="sD")
                stageA = out_pool.tile([TMAIN, 2, V], f16, tag="sA")
                for j in range(NG):
                    u = NG * m + j
                    Wj = W8[:, 128 * j:128 * j + TMAIN]
                    pb = psum_b.tile([128, V], f32, tag="pb")
                    for c2 in range(2):
                        bsl = slice(512 * c2, 512 * (c2 + 1))
                        nc.tensor.matmul(
                            pb[:TMAIN, bsl], Wj, Bb[:, bsl],
                            start=True, stop=True)
                    # epilogue: PSUM->f16 downcast with -lse[:,u] applied as
                    # a per-partition scalar; DVE takes even u's, ACT odd
                    if j % 2 == 0:
                        nc.vector.tensor_scalar_sub(
                            stageD[:, j // 2, :], pb[:TMAIN, :],
                            lse_t32[:TMAIN, u:u + 1])
                    else:
                        nc.scalar.activation(
                            stageA[:, j // 2, :], pb[:TMAIN, :],
                            AF.Identity, bias=neg_lse32[:TMAIN, u:u + 1])
                # refresh this B tile's g-rows for group m+NB
                if m + NB < NGRP:
                    nc.sync.dma_start(
                        Bt[m % NB][96:100, :],
                        G16[NG * (m + NB):NG * (m + NB + 1), :],
                    )
                u0 = NG * m
                nc.sync.dma_start(
                    out_d[0:TMAIN, u0:u0 + NG:2, :], stageD[:])
                nc.sync.dma_start(
                    out_d[0:TMAIN, u0 + 1:u0 + NG:2, :], stageA[:])

                # one leftover t-row (one-hot f_t broadcast, u-partitions;
                # lse added via a K=1 ones-row matmul), interleaved
                if m % lo_every == lo_every // 2 - 1:
                    t = TMAIN + m // lo_every
                    lse_row = L0T[0:1, 128 * (t - TMAIN):128 * (t - TMAIN + 1)]
                    pb2 = psum_b.tile([128, V], f32, tag="pb")
                    onehot = eye16[:, t:t + 1].broadcast_to([128, 128])
                    for c2 in range(2):
                        sl = slice(512 * c2, 512 * (c2 + 1))
                        nc.tensor.matmul(pb2[:, sl], onehot, F16[:, sl],
                                         start=True, stop=False)
                        nc.tensor.matmul(pb2[:, sl], eye16[:], G16[:, sl],
                                         start=False, stop=False)
                        nc.tensor.matmul(pb2[:, sl], lse_row,
                                         ones1[:, sl], start=False, stop=True)
                    stage2 = lo_pool.tile([128, V], f16)
                    nc.vector.tensor_copy(stage2[:, 0:512], pb2[:, 0:512])
                    nc.scalar.activation(stage2[:, 512:], pb2[:, 512:],
                                         AF.Copy)
                    nc.sync.dma_start(out_d[t, :, :], stage2[:])

    bacc.get_activation_tables = _steered_tables
    try:
        nc.compile()
    finally:
        bacc.get_activation_tables = _orig_tables
    _nc_cache[tag] = nc
    return nc


def _consts():
    eye16 = np.eye(128, dtype=np.float16)
    # B-row map: rows 0:96 = F[0:96], rows 96:100 = g-rows,
    # rows 100:128 = F[96:124]
    w8 = np.zeros((128, NG * 128), dtype=np.float16)
    for j in range(NG):
        blk = w8[:, 128 * j:128 * (j + 1)]
        for k in range(96):
            blk[k, k] = 1.0
        for k in range(100, 128):
            blk[k, k - 4] = 1.0
        for t in range(TMAIN):
            blk[96 + j, t] = 1.0
    return eye16, w8


def _in_maps(f, g):
    eye16, w8 = _consts()
    ones_row = np.ones((1, V), dtype=np.float16)
    f16 = f.astype(np.float16)
    g16 = g.astype(np.float16)
    maps = []
    for i in range(NCORES):
        b, h = divmod(i, 2)
        maps.append({
            "f_sh": np.ascontiguousarray(f16[b, h * TSH:(h + 1) * TSH]),
            "g_sh": np.ascontiguousarray(g16[b]),
            "eye16": eye16,
            "w8": w8,
            "ones_row": ones_row,
        })
    return maps


def _gather(results):
    out = np.empty((B, T, U, V), np.float32)
    for i in range(NCORES):
        b, h = divmod(i, 2)
        out[b, h * TSH:(h + 1) * TSH] = np.asarray(
            results[i]["out_sh"], dtype=np.float32)
    return out


def kernel(**inputs):
    from concourse.bass_utils import run_bass_kernel_spmd

    f = np.asarray(inputs["f"], np.float32)
    g = np.asarray(inputs["g"], np.float32)
    nc = _build()
    res = run_bass_kernel_spmd(nc, _in_maps(f, g), core_ids=list(range(NCORES)))
    return _gather(res.results)
